# revision 36
# baseline (speedup 1.0000x reference)
"""CLIP causal attention (B=8, T=1024, E=768, H=12) on 8 TRN2 NeuronCores.

Strategy: pure data-parallel over batch — core b handles x[b] end to end,
no collectives. All compute in transposed space (embed on partitions):

  X' = x_b^T                       [768, 1024]  (host pre-transposed, bf16)
  Q' = Wq^T @ X' (+bq)             [768, 1024]  lhsT = Wq blocks (host-packed)
  K' = Wk^T @ X' (+bk)             [768, 1024]
  V  = X'^T @ Wv_aug (+bv_aug)     [1024, 780]  per head 65 cols: 64 dims +
                                   a ones column (Wv col = 0, bias = 1) that
                                   the PV matmul turns into the softmax denom
  per head PAIR (2nt, 2nt+1) (KQ orientation, j on partitions, i free):
     the two heads' score matmuls have K=64 (head dim) and live on disjoint
     SBUF partition halves (even: 0-63, odd: 64-127), so bass auto-derives
     tile_position (0,0)/(64,0): emitted back-to-back they run CONCURRENTLY
     on the two 64-row halves of the PE array (2x on the QK^T stage).
     Per 512-wide i-chunk: phase A fills one [128,1024] PSUM tile per j-tile
     (cols 0:512 even head, 512:1024 odd head -> different banks), one
     merged exp() + one merged tri-mask per j-tile covers both heads;
     phase B runs the PV matmuls (K=128, full array) off the SBUF P tiles.
     P' = exp(S' * 1/8)  (no max-subtraction: |S'/8| <= ~7, exact-safe)
     causal: skip fully-masked blocks, restrict to valid cols, tri-mask diag
     O_aug[d,i] = sum_j Vaug_h[j,:65]^T @ P'  (row 64 = softmax denominator)
     O'_h = O_aug[0:64] * broadcast(1/denom)
  out^T = Wo^T @ O' (+bo)          [768, 1024]  lhsT = Wo blocks -> transposed
                                   output; host transposes back. Bias is
                                   per-partition -> native tensor_scalar_add.

Phase B of each (pair, i-chunk) "chunk" is software-pipelined into phase A
of the next chunk in emission order: Tile produces a STATIC per-engine
instruction order, so exp-gated PV matmuls must be interleaved with
always-ready work explicitly or the PE stalls at them.

Input DMAs are host-packed into SBUF-layout contiguous tensors and issued
across three hardware queues (sync: X' + Wo, scalar: Wv + Wq/Wk per-block,
gpsimd: consts; output alternates sync/scalar) so descriptor issue doesn't
serialize; V-projection matmuls tick along as X'/Wv k-chunks land (6 parked
PSUM accumulator chains) to keep the HAM clock gate from re-throttling
during the ~20us input stream. Output is bf16 (host upcasts), transposed so
the bias is per-partition. All matmul operands bf16 (fp32 PSUM accumulation);
measured end-to-end rel l2 err vs fp32 reference ~5.5e-3.
"""

import numpy as np
import ml_dtypes

E = 768
T = 1024
B = 8
H = 12
DH = 64
NT = E // 128          # 6 partition-tiles of the embed dim
NJ = T // 128          # 8 partition-tiles of the token dim
SCALE = DH ** -0.5     # folded into the exp() activation's scale operand
VW = H * 65            # V_aug row width: 12 heads x (64 dims + ones col)

_CACHE = {}


def _build():
    import concourse.bass as bass
    import concourse.tile as tile
    from concourse import bacc, mybir

    f32 = mybir.dt.float32
    bf16 = mybir.dt.bfloat16
    Exp = mybir.ActivationFunctionType.Exp

    nc = bacc.Bacc(
        "TRN2",
        target_bir_lowering=False,
        debug=False,
        enable_asserts=False,
        num_devices=B,
    )

    xt = nc.dram_tensor("xt", [128, NT * T], bf16, kind="ExternalInput").ap()
    wq = nc.dram_tensor("wq", [128, NT * E], bf16, kind="ExternalInput").ap()
    wk = nc.dram_tensor("wk", [128, NT * E], bf16, kind="ExternalInput").ap()
    wv = nc.dram_tensor("wv", [128, NT * VW], bf16, kind="ExternalInput").ap()
    wo = nc.dram_tensor("wo", [128, NT * E], bf16, kind="ExternalInput").ap()
    bqt = nc.dram_tensor("bqt", [128, NT], f32, kind="ExternalInput").ap()
    bkt = nc.dram_tensor("bkt", [128, NT], f32, kind="ExternalInput").ap()
    bot = nc.dram_tensor("bot", [128, NT], f32, kind="ExternalInput").ap()
    bvb = nc.dram_tensor("bvb", [128, VW], bf16, kind="ExternalInput").ap()
    tri = nc.dram_tensor("tri", [128, 256], bf16, kind="ExternalInput").ap()
    out_t = nc.dram_tensor("out_t", [E, T], bf16, kind="ExternalOutput").ap()

    with tile.TileContext(nc) as tc:
        with (
            tc.tile_pool(name="const", bufs=1) as cpool,
            tc.tile_pool(name="psb", bufs=12) as ppool,
            tc.tile_pool(name="rsb", bufs=4) as rpool,
            tc.tile_pool(name="rbsb", bufs=4) as rbpool,
            tc.tile_pool(name="fin", bufs=3) as fpool,
            tc.tile_pool(name="pp", bufs=2, space="PSUM") as pp,
            tc.tile_pool(name="sp", bufs=2, space="PSUM") as sp,
            tc.tile_pool(name="op", bufs=2, space="PSUM") as op,
        ):
            XT = cpool.tile([128, NT * T], bf16)     # (kt, i)
            WQ = cpool.tile([128, NT * E], bf16)     # (nt, kt, c): lhsT blocks
            WK = cpool.tile([128, NT * E], bf16)
            WV = cpool.tile([128, NT * VW], bf16)    # (kt, h*65+c); col 64 of
                                                     # each head block = 0
            WO = cpool.tile([128, NT * E], bf16)     # (nt, et, c): lhsT blocks
            QS = cpool.tile([128, NT * T], bf16)     # Q' (nt, i)
            KS = cpool.tile([128, NT * T], bf16)
            VS = cpool.tile([128, NJ * VW], bf16)    # (jt, h*65+d); col 64 of
                                                     # each head block = denom ones
            OS = cpool.tile([128, NT * T], bf16)     # O' (et, i)
            BQ = cpool.tile([128, NT], f32)
            BK = cpool.tile([128, NT], f32)
            BO = cpool.tile([128, NT], f32)
            BVB = cpool.tile([128, VW], bf16)        # bv_aug pre-broadcast (host)
            TRI2 = cpool.tile([128, 256], bf16)      # [tri | tri] for pair masks

            # ---- input DMAs across three queues, priority order. sync: X'
            # per-k-tile (V projection starts as chunks land) then Wo (needed
            # last). scalar: small consts then Wq/Wk. gpsimd: bv_aug + Wv.
            nc.gpsimd.dma_start(BQ[:], bqt)
            nc.gpsimd.dma_start(BK[:], bkt)
            nc.gpsimd.dma_start(BO[:], bot)
            nc.gpsimd.dma_start(TRI2[:], tri)
            nc.gpsimd.dma_start(BVB[:], bvb)
            for kt in range(NT):
                nc.sync.dma_start(XT[:, kt * T : (kt + 1) * T], xt[:, kt * T : (kt + 1) * T])
                nc.scalar.dma_start(WV[:, kt * VW : (kt + 1) * VW], wv[:, kt * VW : (kt + 1) * VW])
            for nt in range(NT):
                nc.scalar.dma_start(WQ[:, nt * E : (nt + 1) * E], wq[:, nt * E : (nt + 1) * E])
                nc.scalar.dma_start(WK[:, nt * E : (nt + 1) * E], wk[:, nt * E : (nt + 1) * E])
            nc.sync.dma_start(WO[:], wo)

            # ---- PE warmup: dummy matmuls with no DMA dependency so the
            # HAM activity monitor lifts the 1.2GHz cold gate before real
            # work arrives (DUM memset first: it gates the dummies) ----
            DUMW = cpool.tile([128, 128], bf16)
            DUMR = cpool.tile([128, 512], bf16)
            nc.vector.memset(DUMW[:], 1.0)
            nc.vector.memset(DUMR[:], 1.0)

            def dummy(n=512):
                # full-array junk matmul: the HAM activity monitor only lifts
                # the 1.2GHz cold gate for real array occupancy.
                d_ps = pp.tile([128, 512], f32, tag="proj")
                nc.tensor.matmul(
                    d_ps[:, :n], lhsT=DUMW[:], rhs=DUMR[:, :n], start=True, stop=True
                )

            for _ in range(30):
                dummy()

            # ---- Q'/K' projection for one 128-row block nt (2 heads) ----
            def qk_proj(nt):
                for W, Bb, DST in ((WQ, BQ, QS), (WK, BK, KS)):
                    for ic in range(2):
                        ps = pp.tile([128, 512], f32, tag="proj")
                        for kt in range(NT):
                            nc.tensor.matmul(
                                ps[:],
                                lhsT=W[:, nt * E + kt * 128 : nt * E + kt * 128 + 128],
                                rhs=XT[:, kt * T + ic * 512 : kt * T + ic * 512 + 512],
                                start=(kt == 0),
                                stop=(kt == NT - 1),
                            )
                        nc.vector.tensor_scalar_add(
                            DST[:, nt * T + ic * 512 : nt * T + ic * 512 + 512],
                            ps[:],
                            Bb[:, nt : nt + 1],
                        )

            # ---- V projection: lhsT = X'[kt, jblk] -> V_aug[j, h*65+c].
            # The ones column comes out of the projection itself (Wv col 0,
            # bias 1), so evictions are plain contiguous adds.
            # The first 5 chains are emitted KT-OUTER: the static PE order
            # would otherwise let chain 0 pace through the chunk-DMA arrivals
            # alone (later chains' ready matmuls sit behind it in program
            # order). Interleaved, each X'/Wv chunk arrival unlocks 5
            # consecutive matmuls; dummies between kt groups absorb arrival
            # jitter so the HAM clock gate never sees a full idle window.
            park = []
            for c in range(5):
                jt, half = c // 2, c % 2
                if c == 0:
                    ps = pp.tile([128, 512], f32, tag="proj", name="vpark")
                elif c < 3:
                    ps = op.tile([128, 512], f32, tag="oaug", name="vpark_o")
                else:
                    ps = sp.tile([128, 512], f32, tag="scores", name="vpark_s")
                park.append((ps, jt, half))
            for kt in range(NT):
                for ps, jt, half in park:
                    e0 = half * 390
                    nc.tensor.matmul(
                        ps[:, :390],
                        lhsT=XT[:, kt * T + jt * 128 : kt * T + jt * 128 + 128],
                        rhs=WV[:, kt * VW + e0 : kt * VW + e0 + 390],
                        start=(kt == 0),
                        stop=(kt == NT - 1),
                        skip_group_check=True,
                    )
                if kt < NT - 1:
                    dummy()
                    dummy()
            for ps, jt, half in park:
                e0 = half * 390
                nc.vector.tensor_add(
                    VS[:, jt * VW + e0 : jt * VW + e0 + 390],
                    ps[:, :390],
                    BVB[:, e0 : e0 + 390],
                )
            # Q/K projection for the first pair goes here, between the
            # parked-chain evictions and the remaining V chains: the DVE is
            # strict FIFO, so emitting its bias-add evictions before the 11
            # remaining V evictions lets the first attention chunk start
            # ~6us earlier.
            qk_proj(0)
            for c in range(5, 2 * NJ):
                jt, half = c // 2, c % 2
                e0 = half * 390
                c6 = c % 6
                if c6 < 2:
                    ps = pp.tile([128, 512], f32, tag="proj")
                elif c6 < 4:
                    ps = op.tile([128, 512], f32, tag="oaug")
                else:
                    ps = sp.tile([128, 512], f32, tag="scores", name="vps")
                for kt in range(NT):
                    nc.tensor.matmul(
                        ps[:, :390],
                        lhsT=XT[:, kt * T + jt * 128 : kt * T + jt * 128 + 128],
                        rhs=WV[:, kt * VW + e0 : kt * VW + e0 + 390],
                        start=(kt == 0),
                        stop=(kt == NT - 1),
                    )
                nc.vector.tensor_add(
                    VS[:, jt * VW + e0 : jt * VW + e0 + 390],
                    ps[:, :390],
                    BVB[:, e0 : e0 + 390],
                )

            def normalize2(o_e, o_o, nt, ic):
                # softmax denominators live in row 64 (the V_aug ones column).
                # Full-precision reciprocal costs 3.35us on DVE; the ~18-bit
                # approx is plenty, but its BITWISE_NOT seed needs an SBUF
                # operand on hardware, so stage the PSUM rows out first.
                # Both heads' denominators share one reciprocal op.
                dn = rpool.tile([1, 1024], f32, tag="denom")
                nc.vector.tensor_copy(dn[0:1, 0:512], o_e[64:65, :])
                nc.vector.tensor_copy(dn[0:1, 512:1024], o_o[64:65, :])
                r = rpool.tile([1, 1024], f32, tag="recip")
                nc.vector.reciprocal_approx_fast(r[:], dn[:])
                for po, o_ps, src in ((0, o_e, r[0:1, 0:512]), (64, o_o, r[0:1, 512:1024])):
                    rb = rbpool.tile([64, 512], f32, tag="recipb")
                    nc.gpsimd.partition_broadcast(rb[:], src)
                    nc.vector.tensor_mul(
                        OS[po : po + 64, nt * T + ic * 512 : nt * T + ic * 512 + 512],
                        o_ps[0:64, :],
                        rb[:],
                    )

            # ---- attention, software-pipelined across (pair, i-chunk)
            # "chunks". Phase A of a chunk: paired scores matmuls (even head
            # -> cols 0:512 = bank A, odd head -> 512:1024 = bank B; disjoint
            # 64-row PE tiles run concurrently), one merged exp + tri-mask
            # per j-tile. Phase B: PV accumulation (full array) off the SBUF
            # P tiles. Phase B of chunk c is interleaved into phase A of
            # chunk c+1 in emission order: its exps are long done, so it is
            # always-ready PE filler that covers the exp pipeline latency
            # (the sp pool only holds 2 score tiles, so S(jt+2) waits on
            # exp(jt); the PV filler keeps the PE busy through that wait,
            # and ScalarE streams exps with no PV-phase idle). ----
            def chunkA(nt, ic, p2s):
                jmax = 4 if ic == 0 else NJ
                for jt in range(jmax):
                    lo = max(0, jt * 128 - ic * 512)
                    s2 = sp.tile([128, 1024], f32, tag="scores")
                    p2 = ppool.tile([128, 1024], bf16, tag="probs")
                    p2s.append((p2, lo))
                    jb = nt * T + jt * 128
                    qlo = nt * T + ic * 512 + lo
                    qhi = nt * T + (ic + 1) * 512
                    nc.tensor.matmul(
                        s2[:, lo:512],
                        lhsT=KS[0:64, jb : jb + 128],
                        rhs=QS[0:64, qlo:qhi],
                        start=True,
                        stop=True,
                    )
                    nc.tensor.matmul(
                        s2[:, 512 + lo : 1024],
                        lhsT=KS[64:128, jb : jb + 128],
                        rhs=QS[64:128, qlo:qhi],
                        start=True,
                        stop=True,
                    )
                    nc.scalar.activation(
                        p2[:].rearrange("p (h c) -> p h c", h=2)[:, :, lo:512],
                        s2[:].rearrange("p (h c) -> p h c", h=2)[:, :, lo:512],
                        Exp,
                        scale=SCALE,
                    )
                    dl = jt * 128 - ic * 512
                    if dl >= 0:  # diagonal block lives in this i-chunk
                        nc.vector.tensor_mul(
                            p2[:].rearrange("p (h c) -> p h c", h=2)[:, :, dl : dl + 128],
                            p2[:].rearrange("p (h c) -> p h c", h=2)[:, :, dl : dl + 128],
                            TRI2[:].rearrange("p (h c) -> p h c", h=2),
                        )
                    yield

            def chunkB(nt, ic, p2s):
                he, ho = 2 * nt, 2 * nt + 1
                jmax = len(p2s)
                o_e = op.tile([128, 512], f32, tag="oaug")
                o_o = op.tile([128, 512], f32, tag="oaug")
                for jt, (p2, lo) in enumerate(p2s):
                    nc.tensor.matmul(
                        o_e[0:65, lo:512],
                        lhsT=VS[:, jt * VW + he * 65 : jt * VW + he * 65 + 65],
                        rhs=p2[:, lo:512],
                        start=(jt == 0),
                        stop=(jt == jmax - 1),
                        skip_group_check=True,
                    )
                    nc.tensor.matmul(
                        o_o[0:65, lo:512],
                        lhsT=VS[:, jt * VW + ho * 65 : jt * VW + ho * 65 + 65],
                        rhs=p2[:, 512 + lo : 1024],
                        start=(jt == 0),
                        stop=(jt == jmax - 1),
                        skip_group_check=True,
                    )
                    yield
                normalize2(o_e, o_o, nt, ic)

            prev, plen = None, 0
            pre_chains = {}
            for nt in range(NT):
                for ic in range(2):
                    if ic == 0 and nt > 0:
                        qk_proj(nt)
                    slots = 4 if ic == 0 else NJ
                    p2s = []
                    done = 0
                    extra = None
                    if nt == NT - 1 and ic == 1:
                        # the last chunk's A phase has no successor-B filler
                        # surplus; thread the first two out-proj chains'
                        # et0..4 legs (ready after pair 4) into its slots.
                        def pre_gen():
                            for pc in range(2):
                                f_pre = pp.tile(
                                    [128, 512], f32, tag="proj", name="pre_ops"
                                )
                                for et in range(NT - 1):
                                    nc.tensor.matmul(
                                        f_pre[:],
                                        lhsT=WO[:, et * 128 : et * 128 + 128],
                                        rhs=OS[:, et * T + pc * 512 : et * T + pc * 512 + 512],
                                        start=(et == 0),
                                        stop=False,
                                        skip_group_check=True,
                                    )
                                    yield
                                pre_chains[(0, pc)] = f_pre

                        extra = pre_gen()
                    for si, _ in enumerate(chunkA(nt, ic, p2s)):
                        if prev is not None:
                            quota = ((si + 1) * plen) // slots
                            while done < quota:
                                next(prev, None)
                                done += 1
                        if extra is not None:
                            next(extra, None)
                            next(extra, None)
                    if extra is not None:
                        for _ in extra:
                            pass
                    if prev is not None:
                        for _ in prev:
                            pass
                    if nt == NT - 1 and ic == 1:
                        # pre-start two more out-projection chains on the
                        # freed scores slots (the pp pair was threaded into
                        # this chunk's A slots as filler); their et5 legs
                        # complete in the final loop after the last
                        # normalize.
                        for pc in (2, 3):
                            f_pre = sp.tile([128, 512], f32, tag="scores", name="pre_ops")
                            for et in range(NT - 1):
                                nc.tensor.matmul(
                                    f_pre[:, :512],
                                    lhsT=WO[:, 768 + et * 128 : 768 + et * 128 + 128],
                                    rhs=OS[:, et * T + (pc - 2) * 512 : et * T + (pc - 2) * 512 + 512],
                                    start=(et == 0),
                                    stop=False,
                                    skip_group_check=True,
                                )
                            pre_chains[(1, pc - 2)] = f_pre
                    prev, plen = chunkB(nt, ic, p2s), len(p2s)
            for _ in prev:
                pass

            # ---- output projection, transposed: out^T = Wo^T @ O' (+bo).
            # lhsT = Wo[et-block, nt-block] (host-packed), rhs = O' i-chunks.
            # Bias is per-partition -> fused into the eviction tensor_scalar.
            # bf16 out_t, host transposes/upcasts.
            # Four chains in flight: 2 from the proj pool + 2 borrowed from
            # the (now idle) scores pool, so et0-4 of several chains overlap
            # the last pair's normalize and the per-chain LDWEIGHTS stalls.
            for nt in range(NT):
                fin = fpool.tile([128, T], bf16, tag="fin")
                for ic in range(2):
                    pre = pre_chains.pop((nt, ic), None)
                    if pre is not None:
                        f_ps, et0 = pre, NT - 1
                    elif (2 * nt + ic) % 2 == 0:
                        f_ps, et0 = pp.tile([128, 512], f32, tag="proj", name="f_ps"), 0
                    else:
                        f_ps, et0 = sp.tile([128, 512], f32, tag="scores", name="f_ps_s"), 0
                    for et in range(et0, NT):
                        nc.tensor.matmul(
                            f_ps[:, :512],
                            lhsT=WO[:, nt * E + et * 128 : nt * E + et * 128 + 128],
                            rhs=OS[:, et * T + ic * 512 : et * T + ic * 512 + 512],
                            start=(et == 0),
                            stop=(et == NT - 1),
                            skip_group_check=True,
                        )
                    nc.vector.tensor_scalar_add(
                        fin[:, ic * 512 : (ic + 1) * 512], f_ps[:, :512], BO[:, nt : nt + 1]
                    )
                    q = nc.sync if ic == 0 else nc.scalar
                    q.dma_start(
                        out_t[nt * 128 : (nt + 1) * 128, ic * 512 : (ic + 1) * 512],
                        fin[:, ic * 512 : (ic + 1) * 512],
                    )

    nc.compile()
    return nc


def _get_nc():
    if "nc" not in _CACHE:
        _CACHE["nc"] = _build()
    return _CACHE["nc"]


def _pack_w(w):
    # [768, 768] -> [128, nt*768 + kt*128 + c] = w[kt*128+p, nt*128+c]
    return np.ascontiguousarray(
        w.reshape(NT, 128, NT, 128).transpose(1, 2, 0, 3).reshape(128, NT * E)
    )


def _make_in_maps(inputs):
    bf = ml_dtypes.bfloat16
    x = np.asarray(inputs["x"], np.float32)
    wv4 = np.asarray(inputs["Wv"], np.float32).reshape(E, H, DH)
    wv_aug = np.zeros((E, H, 65), np.float32)
    wv_aug[:, :, :DH] = wv4
    bv_aug = np.zeros((H, 65), np.float32)
    bv_aug[:, :DH] = np.asarray(inputs["bv"], np.float32).reshape(H, DH)
    bv_aug[:, DH] = 1.0
    shared = {
        "wq": _pack_w(np.asarray(inputs["Wq"], np.float32)).astype(bf),
        "wk": _pack_w(np.asarray(inputs["Wk"], np.float32)).astype(bf),
        "wo": _pack_w(np.asarray(inputs["Wo"], np.float32)).astype(bf),
        "wv": np.ascontiguousarray(
            wv_aug.reshape(NT, 128, VW).transpose(1, 0, 2).reshape(128, NT * VW)
        ).astype(bf),
        "bvb": np.ascontiguousarray(
            np.broadcast_to(bv_aug.reshape(1, VW), (128, VW))
        ).astype(bf),
        "bqt": np.ascontiguousarray(
            np.asarray(inputs["bq"], np.float32).reshape(NT, 128).T
        ),
        "bkt": np.ascontiguousarray(
            np.asarray(inputs["bk"], np.float32).reshape(NT, 128).T
        ),
        "bot": np.ascontiguousarray(
            np.asarray(inputs["bo"], np.float32).reshape(NT, 128).T
        ),
        "tri": np.tile(np.triu(np.ones((128, 128), np.float32)), (1, 2)).astype(bf),
    }
    xs = []
    for b in range(B):
        xp = np.ascontiguousarray(
            x[b].T.reshape(NT, 128, T).transpose(1, 0, 2).reshape(128, NT * T)
        ).astype(bf)
        xs.append(dict(shared, xt=xp))
    return xs


def _run(inputs, trace=False):
    from concourse import bass_utils

    nc = _get_nc()
    res = bass_utils.run_bass_kernel_spmd(
        nc, _make_in_maps(inputs), core_ids=list(range(B)), trace=trace
    )
    out = np.stack(
        [np.asarray(res.results[c]["out_t"]).astype(np.float32).T for c in range(B)]
    )
    return out, res


def kernel(**inputs) -> np.ndarray:
    out, _ = _run(inputs, trace=False)
    return out
